# revision 20
# baseline (speedup 1.0000x reference)
"""GCNConv model Trainium2 Bass kernel.

Architecture (graph-data-parallel over 8 NeuronCores, dst-node sharding at
graph boundaries):

  For each GCN layer (3 layers, then a final linear + pool + MLP head):
    1. transform: h = h1 @ W.T on PE. h1 kept feature-major in SBUF
       ([128 feat, N nodes]); each 128-node column block is the stationary
       operand, W.T streams -> node-major h tiles in PSUM -> SBUF -> HBM "g".
    2. gather: SWDGE dma_gather pulls g[row_e] rows (512B granule) for this
       core's edges, sorted by destination, into edge-major SBUF tiles
       [128 edges, 128 feat].  int16 index limit handled by splitting each
       core's edge stream into two halves by source row (< / >= Npad/2) with
       separate base offsets into g.
    3. scatter: PE matmul-scatter.  gathered tile is stationary lhsT
       [K=128 edges, M=128 feat]; a host-built one-hot-ish matrix S
       [128 edges, span] (S[e, dst-col] = dinv[row_e], the separable half of
       the GCN edge norm) streams as rhs -> PSUM window [128 feat, 512 dst]
       accumulates.  Self-loops are appended as ordinary edges with value
       dinv[n].
    4. evict window: agg * dinvB (dst half of the norm, broadcast from host)
       on DVE, then relu(x + b) on ACT -> feature-major output shard.
  Host concatenates per-core shards (already feature-major) into the next
  layer's h1.  The single SPMD program requirement across the 8 cores is met
  by computing one shared tile schedule from the max per-destination edge
  counts over all cores.

  Layer 3 is fused with the head program: gcn_out linear (node-major),
  mean-pool as a PE matmul against a host one-hot graph indicator with 1/cnt
  folded in, then the MLP/predictor chain entirely feature-major ([*, 32
  graphs] per core).
"""

import os
import sys
import math
import numpy as np

for _p in ("/opt/trn_rl_repo",):
    if _p not in sys.path and os.path.isdir(_p):
        sys.path.insert(0, _p)

import concourse.bass as bass  # noqa: E402
import concourse.mybir as mybir  # noqa: E402
import concourse.tile as tile  # noqa: E402
from concourse import bacc, library_config  # noqa: E402
from concourse.bass_utils import run_bass_kernel_spmd  # noqa: E402

F32 = mybir.dt.float32
I16 = mybir.dt.int16
P = 128
WIN = 512
GATHER_CHUNK = 2048  # edges per dma_gather instruction
CHUNK_TILES = GATHER_CHUNK // P
N_QUEUES = 4  # SWDGE queues; rotating chunks across them ~2.4x's gather BW

# telemetry (test.py reads these)
LAST_EXEC_NS = []
TRACE = bool(int(os.environ.get("GCN_TRACE", "0")))
SIM_MODE = bool(int(os.environ.get("GCN_SIM", "0")))


def _setup_trace_hooks():
    """Best-effort NTFF profiling under axon: register the ctypes hook the
    trimmed antenv image lacks, and keep profile artifacts local."""
    import types
    import contextlib
    from concourse import bass_utils as bu

    try:
        from antenv import axon_hooks  # noqa: F401
    except ImportError:
        mod = types.ModuleType("antenv.axon_hooks")
        _h = [None]
        mod.set_axon_ntff_profile_hook = lambda h: _h.__setitem__(0, h)
        mod.get_axon_ntff_profile_hook = lambda: _h[0]
        sys.modules["antenv.axon_hooks"] = mod
        import antenv
        antenv.axon_hooks = mod
        try:
            from trn_agent_boot.trn_boot import _ntff_profile_via_ctypes
            hook = _ntff_profile_via_ctypes("/opt/axon/libaxon_pjrt.so")
            if hook is not None:
                mod.set_axon_ntff_profile_hook(hook)
        except Exception as e:  # pragma: no cover
            print(f"ntff hook setup failed: {e}")
    # keep artifacts local: no bucket upload in this environment
    bu.upload_artifacts = lambda tmpdir: tmpdir


if TRACE:
    _setup_trace_hooks()


def cdiv(a, b):
    return -(-a // b)


# ----------------------------------------------------------------------------
# host-side preprocessing
# ----------------------------------------------------------------------------

class Prep:
    pass


def preprocess(x, edge_index, batch_index, n_cores):
    pr = Prep()
    N = x.shape[0]
    G = int(batch_index.max()) + 1 if batch_index.shape[0] else 1
    # G must come from mol_features really; caller fixes pr.G
    row = np.asarray(edge_index[0], dtype=np.int64)
    col = np.asarray(edge_index[1], dtype=np.int64)
    bi = np.asarray(batch_index, dtype=np.int64)

    NT = cdiv(N, P)
    if NT % 2:
        NT += 1  # even tile count so the int16 split point is tile aligned
    Npad = NT * P
    assert Npad <= 65536, "int16 two-way split requires Npad/2 <= 32768"
    SPLIT = Npad // 2

    deg = np.bincount(col, minlength=N).astype(np.float64) + 1.0
    dinv = (1.0 / np.sqrt(deg)).astype(np.float32)

    pr.N, pr.NT, pr.Npad, pr.SPLIT = N, NT, Npad, SPLIT
    pr.dinv = dinv
    return pr, row, col, bi


def build_shards(pr, row, col, bi, G, n_cores):
    N = pr.N
    gpc = G // n_cores
    assert gpc * n_cores == G
    graph_start = np.searchsorted(bi, np.arange(G + 1))  # [G+1]; last == N
    shard_start = graph_start[np.arange(n_cores) * gpc]
    shard_end = graph_start[np.arange(1, n_cores + 1) * gpc]
    cw = (shard_end - shard_start).astype(np.int64)
    NW = max(1, int(cdiv(int(cw.max()), WIN)))
    NSH = NW * WIN
    pr.G, pr.gpc = G, gpc
    pr.graph_start = graph_start
    pr.shard_start, pr.shard_end, pr.cw = shard_start, shard_end, cw
    pr.NW, pr.NSH = NW, NSH

    # augmented edge list: original edges + self loops, S value = dinv[row]
    ar = np.concatenate([row, np.arange(N, dtype=np.int64)])
    ac = np.concatenate([col, np.arange(N, dtype=np.int64)])
    av = pr.dinv[ar]
    core_of = np.searchsorted(shard_start, ac, side="right") - 1

    per_core = []  # [core][stream] -> (r, d, v) sorted by d
    cnts = np.zeros((n_cores, 2, NSH), dtype=np.int64)
    for c in range(n_cores):
        m = core_of == c
        r_, d_, v_ = ar[m], ac[m] - shard_start[c], av[m]
        streams = []
        for s in range(2):
            sm = (r_ >= pr.SPLIT) if s else (r_ < pr.SPLIT)
            rs, dsv, vs = r_[sm], d_[sm], v_[sm]
            order = np.argsort(dsv, kind="stable")
            rs, dsv, vs = rs[order], dsv[order], vs[order]
            if s:
                rs = rs - pr.SPLIT
            cnts[c, s] = np.bincount(dsv, minlength=NSH)
            streams.append((rs.astype(np.int64), dsv, vs.astype(np.float32)))
        per_core.append(streams)
    pr.per_core_edges = per_core
    pr.cnts = cnts
    return pr


def build_schedule(pr, n_cores):
    """Shared (across cores) tile schedule per stream.

    Each tile: (window, col0, span, width, s_off) where width==WIN for the
    first tile of each window in stream 0 (full-width start=True matmul),
    else width==span. s_off is the running offset into the concatenated S
    array.
    """
    NW, NSH = pr.NW, pr.NSH
    schedules = []
    for s in range(2):
        cntmax = pr.cnts[:, s, :].max(axis=0)
        tiles = []
        s_off = 0
        for w in range(NW):
            d = w * WIN
            first = True
            while d < (w + 1) * WIN:
                c0 = d
                tot = 0
                while d < (w + 1) * WIN and tot + cntmax[d] <= P:
                    tot += cntmax[d]
                    d += 1
                assert d > c0, (
                    f"destination {d} has {cntmax[d]} edges > {P} on some core"
                )
                span = d - c0
                width = WIN if (first and s == 0) else span
                tiles.append(dict(w=w, col0=c0, span=span, width=width,
                                  s_off=s_off))
                s_off += width
                first = False
        schedules.append(dict(tiles=tiles, s_width=s_off,
                              n_tiles=len(tiles)))
    pr.schedules = schedules
    # per-window tile index ranges per stream
    for s in range(2):
        sch = schedules[s]
        win_ranges = []
        t0 = 0
        for w in range(pr.NW):
            t1 = t0
            while t1 < sch["n_tiles"] and sch["tiles"][t1]["w"] == w:
                t1 += 1
            win_ranges.append((t0, t1))
            t0 = t1
        sch["win_ranges"] = win_ranges
        sch["n_tiles_pad"] = cdiv(sch["n_tiles"], CHUNK_TILES) * CHUNK_TILES
    return pr


def wrap16(idx_flat):
    """Pack an int16 index stream into the [128, n/16] wrapped+replicated
    layout dma_gather expects (idx i lives at partition i%16, col i//16,
    replicated across the 8 16-partition groups)."""
    n = idx_flat.shape[0]
    assert n % 16 == 0
    a = idx_flat.reshape(n // 16, 16).T.astype(np.int16)  # [16, n/16]
    return np.tile(a, (8, 1))  # [128, n/16]


def build_core_data(pr, c):
    """Per-core gather index arrays and S matrices, following the shared
    schedule."""
    out = {}
    for s in range(2):
        sch = pr.schedules[s]
        rs, ds, vs = pr.per_core_edges[c][s]
        ntp = sch["n_tiles_pad"]
        idx = np.zeros(ntp * P, dtype=np.int16)
        S = np.zeros((P, sch["s_width"]), dtype=np.float32)
        # edges sorted by d; per tile take the contiguous run of its dsts
        starts = np.searchsorted(ds, np.arange(pr.NSH + 1))
        for t, ti in enumerate(sch["tiles"]):
            lo = starts[ti["col0"]]
            hi = starts[ti["col0"] + ti["span"]]
            ne = hi - lo
            assert ne <= P
            if ne:
                idx[t * P: t * P + ne] = rs[lo:hi]
                S[np.arange(ne), ti["s_off"] + (ds[lo:hi] - ti["col0"])] = vs[lo:hi]
        out[f"idx{s}"] = wrap16(idx)
        out[f"S{s}"] = S
    # dst-side norm, broadcast across partitions; zero in the padded tail
    dinvB = np.zeros((P, pr.NSH), dtype=np.float32)
    w = int(pr.cw[c])
    dinvB[:, :w] = pr.dinv[pr.shard_start[c]:pr.shard_end[c]][None, :]
    out["dinvB"] = dinvB
    return out


def build_pool_matrix(pr, c):
    """[P, NTS*gpc] one-hot node->graph indicator with 1/count folded in,
    laid out [p, t*gpc + j] for node-tile t, local graph j."""
    NTS = pr.NSH // P
    gpc = pr.gpc
    Sp = np.zeros((P, NTS * gpc), dtype=np.float32)
    g0 = c * gpc
    counts = np.maximum(
        (pr.graph_start[g0 + 1: g0 + gpc + 1] - pr.graph_start[g0: g0 + gpc]),
        1).astype(np.float64)
    # local graph id per local node
    w = int(pr.cw[c])
    bi_loc = np.searchsorted(pr.graph_start[g0: g0 + gpc + 1],
                             np.arange(pr.shard_start[c], pr.shard_end[c]),
                             side="right") - 1
    for n in range(w):
        t, p = divmod(n, P)
        j = bi_loc[n]
        Sp[p, t * gpc + j] = 1.0 / counts[j]
    return Sp


# ----------------------------------------------------------------------------
# device programs
# ----------------------------------------------------------------------------

def _emit_layer(nc, tc, pr, aps, h3_tile=None):
    """Emit transform + gather + scatter + evict for one GCN layer.

    aps: dict of dram APs (h1T, Wt, bvec, dinvB, S0, S1, idx0, idx1, g,
    out_shard or None when h3_tile given).
    If h3_tile is not None the evicted windows are written there (SBUF)
    instead of to HBM.
    """
    NT, NW, NSH = pr.NT, pr.NW, pr.NSH
    f32 = F32

    from contextlib import ExitStack
    stack = ExitStack()
    cpool = stack.enter_context(tc.tile_pool(name="Lconst", bufs=1))
    hpool = stack.enter_context(tc.tile_pool(name="Lh1", bufs=2))
    ptpool = stack.enter_context(tc.tile_pool(name="LpsumT", bufs=3,
                                              space="PSUM"))
    gspool = stack.enter_context(tc.tile_pool(name="Lgstage", bufs=3))
    ipool = stack.enter_context(tc.tile_pool(name="Lidx", bufs=1))
    gapool = stack.enter_context(tc.tile_pool(name="Lgath", bufs=3))
    spool = stack.enter_context(tc.tile_pool(name="Lswin", bufs=2))
    pwpool = stack.enter_context(tc.tile_pool(name="LpsumW", bufs=3,
                                              space="PSUM"))
    epool = stack.enter_context(tc.tile_pool(name="Levict", bufs=3))

    Wt_t = cpool.tile([P, P], f32, tag="wt")
    nc.sync.dma_start(Wt_t[:], aps["Wt"])
    bvec_t = cpool.tile([P, 1], f32, tag="bv")
    nc.sync.dma_start(bvec_t[:], aps["bvec"])
    dinvB_t = cpool.tile([P, NSH], f32, tag="dinvB")
    nc.sync.dma_start(dinvB_t[:], aps["dinvB"])

    idx_t = []
    for s in range(2):
        ntp = pr.schedules[s]["n_tiles_pad"]
        t = ipool.tile([P, ntp * 8], I16, tag=f"idx{s}")
        nc.sync.dma_start(t[:], aps[f"idx{s}"])
        idx_t.append(t)

    # ---- transform: h = h1 @ W.T, node-major out -> g in HBM -------------
    g_ap = aps["g"]
    TCH = 32  # node tiles per h1 chunk
    for ch in range(cdiv(NT, TCH)):
        t0 = ch * TCH
        ntile = min(TCH, NT - t0)
        hc = hpool.tile([P, TCH * P], f32, tag="h1c")
        nc.sync.dma_start(hc[:, : ntile * P],
                          aps["h1T"][:, t0 * P: (t0 + ntile) * P])
        for q in range(0, ntile, 4):
            ngrp = min(4, ntile - q)
            pt = ptpool.tile([P, 4 * P], f32, tag="ptrans")
            for j in range(ngrp):
                nc.tensor.matmul(
                    out=pt[:, j * P: (j + 1) * P],
                    lhsT=hc[:, (q + j) * P: (q + j + 1) * P],
                    rhs=Wt_t[:],
                    start=True, stop=True)
            st = gspool.tile([P, 4 * P], f32, tag="gst")
            nc.vector.tensor_copy(out=st[:, : ngrp * P], in_=pt[:, : ngrp * P])
            # SBUF [p, j, f] -> HBM g[(t0+q+j)*128 + p, f]
            dst = g_ap[(t0 + q) * P: (t0 + q + ngrp) * P, :]
            dst = dst.rearrange("(j p) f -> p j f", p=P)
            nc.sync.dma_start(dst, st[:, : ngrp * P].rearrange(
                "p (j f) -> p j f", f=P))

    # ---- gather + matmul scatter + evict ---------------------------------
    halves = [aps["g"][0: pr.SPLIT, :], aps["g"][pr.SPLIT: pr.Npad, :]]
    gath_tiles = [{}, {}]

    qctr = [0]

    def chunk_tile(s, cidx):
        if cidx not in gath_tiles[s]:
            t = gapool.tile([P, CHUNK_TILES, P], f32, tag="gchunk")
            if os.environ.get("GCN_NO_GATHER"):
                nc.vector.memset(t[:], 0.0)
            else:
                nc.gpsimd.dma_gather(
                    t[:],
                    halves[s],
                    idx_t[s][:, cidx * (GATHER_CHUNK // 16):
                             (cidx + 1) * (GATHER_CHUNK // 16)],
                    GATHER_CHUNK, GATHER_CHUNK, P,
                    single_packet=(GATHER_CHUNK <= 512),
                    queue_num=qctr[0] % N_QUEUES)
                qctr[0] += 1
            gath_tiles[s][cidx] = t
        return gath_tiles[s][cidx]

    for w in range(NW):
        pw = pwpool.tile([P, WIN], f32, tag="pwin")
        s_sb = []
        for s in range(2):
            sch = pr.schedules[s]
            tr = sch["win_ranges"][w]
            w_lo = sch["tiles"][tr[0]]["s_off"]
            w_hi = (sch["tiles"][tr[1] - 1]["s_off"] +
                    sch["tiles"][tr[1] - 1]["width"]) if tr[1] > tr[0] else w_lo
            wwidth = w_hi - w_lo
            st = spool.tile([P, pr.max_swin[s]], f32, tag=f"swin{s}")
            if wwidth:
                nc.sync.dma_start(st[:, :wwidth],
                                  aps[f"S{s}"][:, w_lo:w_hi])
            s_sb.append((st, w_lo))

        first = True
        n_mm = sum(pr.schedules[s]["win_ranges"][w][1] -
                   pr.schedules[s]["win_ranges"][w][0] for s in range(2))
        mm_i = 0
        for s in range(2):
            sch = pr.schedules[s]
            st, w_lo = s_sb[s]
            for t in range(*sch["win_ranges"][w]):
                ti = sch["tiles"][t]
                gt = chunk_tile(s, t // CHUNK_TILES)
                lhsT = gt[:, t % CHUNK_TILES, :]
                rhs = st[:, ti["s_off"] - w_lo: ti["s_off"] - w_lo + ti["width"]]
                out = pw[:, ti["col0"] - w * WIN:
                         ti["col0"] - w * WIN + ti["width"]]
                mm_i += 1
                nc.tensor.matmul(out=out, lhsT=lhsT, rhs=rhs,
                                 start=first, stop=(mm_i == n_mm),
                                 skip_group_check=True)
                first = False

        ev = epool.tile([P, WIN], f32, tag="ev")
        nc.vector.tensor_mul(out=ev[:], in0=pw[:],
                             in1=dinvB_t[:, w * WIN: (w + 1) * WIN])
        if h3_tile is not None:
            nc.scalar.activation(out=h3_tile[:, w * WIN: (w + 1) * WIN],
                                 in_=ev[:],
                                 func=mybir.ActivationFunctionType.Relu,
                                 bias=bvec_t[:, 0:1])
        else:
            ev2 = epool.tile([P, WIN], f32, tag="ev2")
            nc.scalar.activation(out=ev2[:], in_=ev[:],
                                 func=mybir.ActivationFunctionType.Relu,
                                 bias=bvec_t[:, 0:1])
            nc.sync.dma_start(aps["out_shard"][:, w * WIN: (w + 1) * WIN],
                              ev2[:])

    stack.close()


def build_layer_program(pr, n_cores):
    nc = bacc.Bacc("TRN2", target_bir_lowering=False, debug=False,
                   num_devices=n_cores, num_swdge_queues=N_QUEUES)
    f32 = F32
    aps = {}
    aps["h1T"] = nc.dram_tensor("h1T", [P, pr.Npad], f32,
                                kind="ExternalInput").ap()
    aps["Wt"] = nc.dram_tensor("Wt", [P, P], f32, kind="ExternalInput").ap()
    aps["bvec"] = nc.dram_tensor("bvec", [P, 1], f32,
                                 kind="ExternalInput").ap()
    aps["dinvB"] = nc.dram_tensor("dinvB", [P, pr.NSH], f32,
                                  kind="ExternalInput").ap()
    for s in range(2):
        sch = pr.schedules[s]
        aps[f"S{s}"] = nc.dram_tensor(f"S{s}", [P, sch["s_width"]], f32,
                                      kind="ExternalInput").ap()
        aps[f"idx{s}"] = nc.dram_tensor(f"idx{s}", [P, sch["n_tiles_pad"] * 8],
                                        I16, kind="ExternalInput").ap()
    aps["g"] = nc.dram_tensor("g", [pr.Npad, P], f32, kind="Internal").ap()
    aps["out_shard"] = nc.dram_tensor("out_shard", [P, pr.NSH], f32,
                                      kind="ExternalOutput").ap()

    with tile.TileContext(nc) as tc:
        nc.gpsimd.load_library(library_config.mlp)
        _emit_layer(nc, tc, pr, aps)
    nc.compile()
    return nc  # noqa


def build_final_program(pr, n_cores):
    """Layer 3 + gcn_out linear + mean pool + MLP + predictor head."""
    nc = bacc.Bacc("TRN2", target_bir_lowering=False, debug=False,
                   num_devices=n_cores, num_swdge_queues=N_QUEUES)
    f32 = F32
    gpc = pr.gpc
    NTS = pr.NSH // P
    aps = {}
    aps["h1T"] = nc.dram_tensor("h1T", [P, pr.Npad], f32,
                                kind="ExternalInput").ap()
    aps["Wt"] = nc.dram_tensor("Wt", [P, P], f32, kind="ExternalInput").ap()
    aps["bvec"] = nc.dram_tensor("bvec", [P, 1], f32,
                                 kind="ExternalInput").ap()
    aps["dinvB"] = nc.dram_tensor("dinvB", [P, pr.NSH], f32,
                                  kind="ExternalInput").ap()
    for s in range(2):
        sch = pr.schedules[s]
        aps[f"S{s}"] = nc.dram_tensor(f"S{s}", [P, sch["s_width"]], f32,
                                      kind="ExternalInput").ap()
        aps[f"idx{s}"] = nc.dram_tensor(f"idx{s}", [P, sch["n_tiles_pad"] * 8],
                                        I16, kind="ExternalInput").ap()
    aps["g"] = nc.dram_tensor("g", [pr.Npad, P], f32, kind="Internal").ap()

    # head weights (all pre-transposed host side to [fan_in, fan_out])
    aps["WoT"] = nc.dram_tensor("WoT", [P, P], f32, kind="ExternalInput").ap()
    aps["boB"] = nc.dram_tensor("boB", [P, P], f32, kind="ExternalInput").ap()
    aps["Spool"] = nc.dram_tensor("Spool", [P, NTS * gpc], f32,
                                  kind="ExternalInput").ap()
    aps["molT"] = nc.dram_tensor("molT", [256, gpc], f32,
                                 kind="ExternalInput").ap()
    aps["W1T"] = nc.dram_tensor("W1T", [256, 256], f32,
                                kind="ExternalInput").ap()
    aps["b1"] = nc.dram_tensor("b1", [P, 2], f32, kind="ExternalInput").ap()
    aps["W2T"] = nc.dram_tensor("W2T", [256, 256], f32,
                                kind="ExternalInput").ap()
    aps["b2"] = nc.dram_tensor("b2", [P, 2], f32, kind="ExternalInput").ap()
    aps["WmoT"] = nc.dram_tensor("WmoT", [256, 64], f32,
                                 kind="ExternalInput").ap()
    aps["bmo"] = nc.dram_tensor("bmo", [64, 1], f32,
                                kind="ExternalInput").ap()
    aps["P1T"] = nc.dram_tensor("P1T", [192, 256], f32,
                                kind="ExternalInput").ap()
    aps["pb1"] = nc.dram_tensor("pb1", [P, 2], f32, kind="ExternalInput").ap()
    aps["P2T"] = nc.dram_tensor("P2T", [256, 256], f32,
                                kind="ExternalInput").ap()
    aps["pb2"] = nc.dram_tensor("pb2", [P, 2], f32, kind="ExternalInput").ap()
    aps["oT"] = nc.dram_tensor("oT", [256, 1], f32,
                               kind="ExternalInput").ap()
    aps["ob"] = nc.dram_tensor("ob", [1, 1], f32, kind="ExternalInput").ap()
    aps["pred"] = nc.dram_tensor("pred", [1, gpc], f32,
                                 kind="ExternalOutput").ap()

    Relu = mybir.ActivationFunctionType.Relu
    Copy = mybir.ActivationFunctionType.Copy

    from contextlib import ExitStack
    with tile.TileContext(nc) as tc, ExitStack() as hstack:
        nc.gpsimd.load_library(library_config.mlp)
        perpool = hstack.enter_context(tc.tile_pool(name="Fpersist", bufs=1))
        h3 = perpool.tile([P, pr.NSH], f32, tag="h3")
        _emit_layer(nc, tc, pr, aps, h3_tile=h3)

        hcpool = hstack.enter_context(tc.tile_pool(name="Fhconst", bufs=1))
        hppool = hstack.enter_context(tc.tile_pool(name="Fhpsum", bufs=2,
                                                   space="PSUM"))
        hspool = hstack.enter_context(tc.tile_pool(name="Fhsbuf", bufs=3))

        WoT_t = hcpool.tile([P, P], f32, tag="WoT")
        nc.sync.dma_start(WoT_t[:], aps["WoT"])
        boB_t = hcpool.tile([P, P], f32, tag="boB")
        nc.sync.dma_start(boB_t[:], aps["boB"])
        Spool_t = hcpool.tile([P, NTS * gpc], f32, tag="Spool")
        nc.sync.dma_start(Spool_t[:], aps["Spool"])

        # gcn_out transform (node-major out) + mean pool matmul
        pool_ps = hppool.tile([P, gpc], f32, tag="poolps")
        for t in range(NTS):
            hp = hppool.tile([P, P], f32, tag="houtps")
            nc.tensor.matmul(out=hp[:], lhsT=h3[:, t * P: (t + 1) * P],
                             rhs=WoT_t[:], start=True, stop=True)
            hs = hspool.tile([P, P], f32, tag="houts")
            nc.vector.tensor_add(out=hs[:], in0=hp[:], in1=boB_t[:])
            nc.tensor.matmul(out=pool_ps[:], lhsT=hs[:],
                             rhs=Spool_t[:, t * gpc: (t + 1) * gpc],
                             start=(t == 0), stop=(t == NTS - 1))
        poolT = hcpool.tile([P, gpc], f32, tag="poolT")
        nc.scalar.activation(out=poolT[:], in_=pool_ps[:], func=Copy)

        # --- feature-major head, [*, gpc] ---------------------------------
        def load_w(name, rows, cols):
            tls = []
            for h in range(cdiv(rows, P)):
                r = min(P, rows - h * P)
                t = hcpool.tile([P, cols], f32, tag=f"{name}_{h}")
                nc.sync.dma_start(t[:r, :], aps[name][h * P: h * P + r, :])
                tls.append((t, r))
            return tls

        def load_b(name, rows):
            nh = cdiv(rows, P)
            t = hcpool.tile([P, nh], f32, tag=f"{name}_b")
            nc.sync.dma_start(t[:min(P, rows), :nh], aps[name])
            return t

        molT = load_w("molT", 256, gpc)
        W1T = load_w("W1T", 256, 256)
        b1 = load_b("b1", 256)
        W2T = load_w("W2T", 256, 256)
        b2 = load_b("b2", 256)
        WmoT = load_w("WmoT", 256, 64)
        bmo = load_b("bmo", 64)
        P1T = load_w("P1T", 192, 256)
        pb1 = load_b("pb1", 256)
        P2T = load_w("P2T", 256, 256)
        pb2 = load_b("pb2", 256)
        oT = load_w("oT", 256, 1)
        ob = load_b("ob", 1)

        def dense(x_parts, wts, bias_t, fan_out, act):
            """x_parts: list of (tile, rows) feature-major activations.
            wts: list of (tile, rows) chunks of [fan_in, fan_out] weights.
            Returns list of (tile, rows) output chunks."""
            outs = []
            for h in range(cdiv(fan_out, P)):
                fo = min(P, fan_out - h * P)
                ps = hppool.tile([P, gpc], f32, tag="headps")
                for k, (xt, xr) in enumerate(x_parts):
                    wt, wr = wts[k]
                    assert wr == xr
                    nc.tensor.matmul(
                        out=ps[:fo, :],
                        lhsT=wt[:xr, h * P: h * P + fo],
                        rhs=xt[:xr, :],
                        start=(k == 0), stop=(k == len(x_parts) - 1))
                ot = hspool.tile([P, gpc], f32, tag="heados")
                if act is None:  # bias only, no nonlinearity
                    nc.vector.tensor_scalar_add(out=ot[:fo, :],
                                                in0=ps[:fo, :],
                                                scalar1=bias_t[:fo, h: h + 1])
                else:
                    nc.scalar.activation(out=ot[:fo, :], in_=ps[:fo, :],
                                         func=act,
                                         bias=bias_t[:fo, h: h + 1])
                outs.append((ot, fo))
            return outs

        h2 = dense(molT, W1T, b1, 256, Relu)
        h2 = dense(h2, W2T, b2, 256, Relu)
        h2 = dense(h2, WmoT, bmo, 64, Relu)
        cat = [(poolT, P), h2[0]]
        p1 = dense(cat, P1T, pb1, 256, Relu)
        p2 = dense(p1, P2T, pb2, 256, Relu)
        o = dense(p2, oT, ob, 1, None)
        nc.sync.dma_start(aps["pred"], o[0][0][:1, :])
    nc.compile()
    return nc


# ----------------------------------------------------------------------------
# launch helpers
# ----------------------------------------------------------------------------

_RUNNERS = {}


def _build_runner(nc, n_cores):
    """AOT-compiled shard_map executable for `nc` (adapted from
    bass2jax.run_bass_via_pjrt, but compiled once and reusable)."""
    import jax
    from jax.experimental.shard_map import shard_map
    from jax.sharding import Mesh, PartitionSpec
    from concourse import bass2jax, mybir as mb

    bass2jax.install_neuronx_cc_hook()
    partition_name = (nc.partition_id_tensor.name
                      if nc.partition_id_tensor else None)
    in_names, out_names, out_avals, zero_shapes = [], [], [], []
    for alloc in nc.m.functions[0].allocations:
        if not isinstance(alloc, mb.MemoryLocationSet):
            continue
        name = alloc.memorylocations[0].name
        if alloc.kind == "ExternalInput":
            if name != partition_name:
                in_names.append(name)
        elif alloc.kind == "ExternalOutput":
            shape = tuple(alloc.tensor_shape)
            dtype = mb.dt.np(alloc.dtype)
            out_names.append(name)
            out_avals.append(jax.core.ShapedArray(shape, dtype))
            zero_shapes.append((shape, dtype))
    n_params = len(in_names)
    n_outs = len(out_avals)
    all_in_names = list(in_names) + list(out_names)
    if partition_name is not None:
        all_in_names.append(partition_name)
    donate = tuple(range(n_params, n_params + n_outs))

    def _body(*args):
        operands = list(args)
        if partition_name is not None:
            operands.append(bass2jax.partition_id_tensor())
        outs = bass2jax._bass_exec_p.bind(
            *operands,
            out_avals=tuple(out_avals),
            in_names=tuple(all_in_names),
            out_names=tuple(out_names),
            lowering_input_output_aliases=(),
            sim_require_finite=True,
            sim_require_nnan=True,
            nc=nc,
        )
        return tuple(outs)

    devices = jax.devices()[:n_cores]
    mesh = Mesh(np.asarray(devices), ("core",))
    in_specs = (PartitionSpec("core"),) * (n_params + n_outs)
    out_specs = (PartitionSpec("core"),) * n_outs
    jitted = jax.jit(
        shard_map(_body, mesh=mesh, in_specs=in_specs, out_specs=out_specs,
                  check_rep=False),
        donate_argnums=donate, keep_unused=True)

    # abstract args for AOT lowering (global = n_cores * per-core on axis 0)
    abstract = []
    shp = {}
    for alloc in nc.m.functions[0].allocations:
        if isinstance(alloc, mb.MemoryLocationSet) and alloc.kind in (
                "ExternalInput", "ExternalOutput"):
            shp[alloc.memorylocations[0].name] = (
                tuple(alloc.tensor_shape), mb.dt.np(alloc.dtype))
    for name in in_names:
        s, d = shp[name]
        abstract.append(jax.ShapeDtypeStruct((n_cores * s[0],) + s[1:], d))
    for s, d in zero_shapes:
        abstract.append(jax.ShapeDtypeStruct((n_cores * s[0],) + s[1:], d))
    compiled = jitted.lower(*abstract).compile()
    return dict(compiled=compiled, in_names=in_names, out_names=out_names,
                zero_shapes=zero_shapes, out_avals=out_avals)


def _run_launch(nc, in_maps, n_cores):
    if SIM_MODE:
        from concourse.bass_interp import MultiCoreSim
        sim = MultiCoreSim(nc, num_cores=n_cores)
        for cid, core in sim.cores.items():
            for k, v in in_maps[cid].items():
                core.tensor(k)[:] = v
        sim.simulate(check_with_hw=False)
        outs = []
        for cid in range(n_cores):
            core = sim.cores[cid]
            o = {}
            for alloc in nc.m.functions[0].allocations:
                if getattr(alloc, "kind", None) == "ExternalOutput":
                    name = alloc.memorylocations[0].name
                    o[name] = np.array(core.tensor(name))
            outs.append(o)
        return outs

    key = id(nc)
    if key not in _RUNNERS:
        _RUNNERS[key] = _build_runner(nc, n_cores)
    r = _RUNNERS[key]
    concat_in = [np.concatenate([np.asarray(in_maps[c][name])
                                 for c in range(n_cores)], axis=0)
                 for name in r["in_names"]]
    concat_zeros = [np.zeros((n_cores * s[0],) + s[1:], d)
                    for s, d in r["zero_shapes"]]

    def _exec():
        out = r["compiled"](*concat_in, *concat_zeros)
        import jax
        jax.block_until_ready(out)
        return out

    if TRACE:
        import tempfile
        from antenv.axon_hooks import get_axon_ntff_profile_hook
        hook = get_axon_ntff_profile_hook()
        tmpdir = tempfile.mkdtemp(prefix="gcn_ntff_")
        try:
            with hook(tmpdir, list(range(n_cores))):
                out_arrs = _exec()
            ns = _extract_exec_ns(nc, tmpdir, n_cores)
            if ns is not None:
                LAST_EXEC_NS.append(ns)
        except Exception as e:
            print(f"traced exec failed ({type(e).__name__}: {e}); "
                  "running untraced")
            out_arrs = _exec()
    else:
        out_arrs = _exec()

    outs = []
    for c in range(n_cores):
        o = {}
        for i, name in enumerate(r["out_names"]):
            a = np.asarray(out_arrs[i])
            per = a.reshape(n_cores, *r["out_avals"][i].shape)
            o[name] = per[c]
        outs.append(o)
    return outs


def _extract_exec_ns(nc, neff_dir, n_cores):
    """ntff -> perfetto -> max per-core exec time (same pipeline as
    bass_utils' axon trace path, kept local)."""
    try:
        import glob as _glob
        from concourse import bass_utils as bu
        import gauge.profiler
        from concourse._compat import FishPath
        ntffs = _glob.glob(os.path.join(neff_dir, "*_body*.ntff"))
        if not ntffs:
            print(f"no ntffs in {neff_dir}: {sorted(os.listdir(neff_dir))}")
            return None
        profile = gauge.profiler.Profile(
            profile_path=FishPath(neff_dir),
            kernel_dev_mode=True,
            profile_on_exit=False,
            bass_kernel=nc.m,
            offline_processing=True,
            fname="*_body*",
            metadata={"artifacts_path": neff_dir},
        )
        pres = bu._process_ntff_profile(
            profile, neff_dir, nc, list(range(n_cores)),
            list(range(n_cores)), False, {}, trace_events=False)
        print(f"  trace dir: {neff_dir}")
        return pres.exec_time_ns
    except Exception as e:
        import traceback
        traceback.print_exc()
        print(f"ntff processing failed: {e}")
        return None


def _layer_inmaps(pr, core_data, h1T, W, b, n_cores):
    Wt = np.ascontiguousarray(W.T).astype(np.float32)
    bvec = np.ascontiguousarray(b.reshape(P, 1)).astype(np.float32)
    maps = []
    for c in range(n_cores):
        cd = core_data[c]
        maps.append(dict(h1T=h1T, Wt=Wt, bvec=bvec, dinvB=cd["dinvB"],
                         S0=cd["S0"], S1=cd["S1"],
                         idx0=cd["idx0"], idx1=cd["idx1"]))
    return maps


def _assemble(pr, outs, n_cores):
    h1T = np.zeros((P, pr.Npad), dtype=np.float32)
    off = 0
    for c in range(n_cores):
        w = int(pr.cw[c])
        h1T[:, off:off + w] = outs[c]["out_shard"][:, :w]
        off += w
    return h1T


# program cache (avoid rebuilding for repeated kernel() calls)
_CACHE = {}


def gcn_forward(inputs, n_cores=8):
    if not SIM_MODE:
        _axon_reset()
    x = np.asarray(inputs["x"], dtype=np.float32)
    edge_index = np.asarray(inputs["edge_index"])
    batch_index = np.asarray(inputs["batch_index"])
    mol = np.asarray(inputs["mol_features"], dtype=np.float32)
    G = mol.shape[0]

    pr, row, col, bi = preprocess(x, edge_index, batch_index, n_cores)
    build_shards(pr, row, col, bi, G, n_cores)
    build_schedule(pr, n_cores)
    pr.max_swin = []
    for s in range(2):
        sch = pr.schedules[s]
        widths = []
        for w in range(pr.NW):
            lo, hi = sch["win_ranges"][w]
            widths.append(sum(t["width"] for t in sch["tiles"][lo:hi]))
        pr.max_swin.append(max(max(widths), 1))

    core_data = [build_core_data(pr, c) for c in range(n_cores)]

    key = (pr.Npad, pr.NSH, tuple(s["s_width"] for s in pr.schedules),
           tuple(s["n_tiles_pad"] for s in pr.schedules),
           tuple(tuple((t["col0"], t["width"], t["w"]) for t in s["tiles"])
                 for s in pr.schedules), G)
    if key not in _CACHE:
        _CACHE.clear()
        nc1 = build_layer_program(pr, n_cores)
        nc2 = build_final_program(pr, n_cores)
        _CACHE[key] = (nc1, nc2)
    nc1, nc2 = _CACHE[key]

    # layer 1+2 on program 1
    h1T = np.zeros((P, pr.Npad), dtype=np.float32)
    h1T[:, :pr.N] = np.ascontiguousarray(x.T)
    gcn_W = np.asarray(inputs["gcn_W"], dtype=np.float32)
    gcn_b = np.asarray(inputs["gcn_b"], dtype=np.float32)
    for L in range(2):
        maps = _layer_inmaps(pr, core_data, h1T, gcn_W[L], gcn_b[L], n_cores)
        outs = _run_launch(nc1, maps, n_cores)
        h1T = _assemble(pr, outs, n_cores)

    # layer 3 + head on program 2
    maps = _layer_inmaps(pr, core_data, h1T, gcn_W[2], gcn_b[2], n_cores)
    gpc = pr.gpc
    for c in range(n_cores):
        m = maps[c]
        m["WoT"] = np.ascontiguousarray(
            np.asarray(inputs["gcn_out_W"], dtype=np.float32).T)
        m["boB"] = np.tile(np.asarray(inputs["gcn_out_b"], dtype=np.float32),
                           (P, 1))
        m["Spool"] = build_pool_matrix(pr, c)
        m["molT"] = np.ascontiguousarray(
            mol[c * gpc:(c + 1) * gpc, :].T).astype(np.float32)
        mlp_W = np.asarray(inputs["mlp_W"], dtype=np.float32)
        mlp_b = np.asarray(inputs["mlp_b"], dtype=np.float32)
        m["W1T"] = np.ascontiguousarray(mlp_W[0].T)
        m["b1"] = np.ascontiguousarray(mlp_b[0].reshape(2, P).T)
        m["W2T"] = np.ascontiguousarray(mlp_W[1].T)
        m["b2"] = np.ascontiguousarray(mlp_b[1].reshape(2, P).T)
        m["WmoT"] = np.ascontiguousarray(
            np.asarray(inputs["mlp_out_W"], dtype=np.float32).T)
        m["bmo"] = np.asarray(inputs["mlp_out_b"],
                              dtype=np.float32).reshape(64, 1)
        m["P1T"] = np.ascontiguousarray(
            np.asarray(inputs["pred_W1"], dtype=np.float32).T)
        m["pb1"] = np.ascontiguousarray(
            np.asarray(inputs["pred_b1"], dtype=np.float32).reshape(2, P).T)
        m["P2T"] = np.ascontiguousarray(
            np.asarray(inputs["pred_W2"], dtype=np.float32).T)
        m["pb2"] = np.ascontiguousarray(
            np.asarray(inputs["pred_b2"], dtype=np.float32).reshape(2, P).T)
        m["oT"] = np.ascontiguousarray(
            np.asarray(inputs["out_W"], dtype=np.float32).T)
        m["ob"] = np.asarray(inputs["out_b"], dtype=np.float32).reshape(1, 1)
    outs = _run_launch(nc2, maps, n_cores)

    pred = np.concatenate([outs[c]["pred"][0] for c in range(n_cores)])
    return pred.reshape(G, 1).astype(np.float32)


def _axon_reset():
    try:
        import ctypes
        import jax
        jax.devices()  # client must be initialized for reset to reach it
        lib = ctypes.CDLL("/opt/axon/libaxon_pjrt.so")
        lib.axon_reset.restype = ctypes.c_int64
        return lib.axon_reset()
    except Exception:
        return None


def kernel(**inputs):
    LAST_EXEC_NS.clear()
    return gcn_forward(inputs, n_cores=8)


# revision 25
# speedup vs baseline: 1.2451x; 1.2451x over previous
"""GCNConv model Trainium2 Bass kernel.

Architecture (graph-data-parallel over 8 NeuronCores, dst-node sharding at
graph boundaries):

  For each GCN layer (3 layers, then a final linear + pool + MLP head):
    1. transform: h = h1 @ W.T on PE. h1 kept feature-major in SBUF
       ([128 feat, N nodes]); each 128-node column block is the stationary
       operand, W.T streams -> node-major h tiles in PSUM -> SBUF -> HBM "g".
    2. gather: SWDGE dma_gather pulls g[row_e] rows (512B granule) for this
       core's edges, sorted by destination, into edge-major SBUF tiles
       [128 edges, 128 feat].  int16 index limit handled by splitting each
       core's edge stream into two halves by source row (< / >= Npad/2) with
       separate base offsets into g.
    3. scatter: PE matmul-scatter.  gathered tile is stationary lhsT
       [K=128 edges, M=128 feat]; a host-built one-hot-ish matrix S
       [128 edges, span] (S[e, dst-col] = dinv[row_e], the separable half of
       the GCN edge norm) streams as rhs -> PSUM window [128 feat, 512 dst]
       accumulates.  Self-loops are appended as ordinary edges with value
       dinv[n].
    4. evict window: agg * dinvB (dst half of the norm, broadcast from host)
       on DVE, then relu(x + b) on ACT -> feature-major output shard.
  Host concatenates per-core shards (already feature-major) into the next
  layer's h1.  The single SPMD program requirement across the 8 cores is met
  by computing one shared tile schedule from the max per-destination edge
  counts over all cores.

  Layer 3 is fused with the head program: gcn_out linear (node-major),
  mean-pool as a PE matmul against a host one-hot graph indicator with 1/cnt
  folded in, then the MLP/predictor chain entirely feature-major ([*, 32
  graphs] per core).
"""

import os
import sys
import math
import ml_dtypes
import numpy as np

BF16_NP = ml_dtypes.bfloat16

for _p in ("/opt/trn_rl_repo",):
    if _p not in sys.path and os.path.isdir(_p):
        sys.path.insert(0, _p)

import concourse.bass as bass  # noqa: E402
import concourse.mybir as mybir  # noqa: E402
import concourse.tile as tile  # noqa: E402
from concourse import bacc, library_config  # noqa: E402
from concourse.bass_utils import run_bass_kernel_spmd  # noqa: E402

F32 = mybir.dt.float32
BF16 = mybir.dt.bfloat16
I16 = mybir.dt.int16
P = 128
WIN = 512
GATHER_CHUNK = 2048  # edges per dma_gather instruction
CHUNK_TILES = GATHER_CHUNK // P
N_QUEUES = 4  # SWDGE queues; rotating chunks across them ~2.4x's gather BW

# telemetry (test.py reads these)
LAST_EXEC_NS = []
TRACE = bool(int(os.environ.get("GCN_TRACE", "0")))
SIM_MODE = bool(int(os.environ.get("GCN_SIM", "0")))


def _setup_trace_hooks():
    """Best-effort NTFF profiling under axon: register the ctypes hook the
    trimmed antenv image lacks, and keep profile artifacts local."""
    import types
    import contextlib
    from concourse import bass_utils as bu

    try:
        from antenv import axon_hooks  # noqa: F401
    except ImportError:
        mod = types.ModuleType("antenv.axon_hooks")
        _h = [None]
        mod.set_axon_ntff_profile_hook = lambda h: _h.__setitem__(0, h)
        mod.get_axon_ntff_profile_hook = lambda: _h[0]
        sys.modules["antenv.axon_hooks"] = mod
        import antenv
        antenv.axon_hooks = mod
        try:
            from trn_agent_boot.trn_boot import _ntff_profile_via_ctypes
            hook = _ntff_profile_via_ctypes("/opt/axon/libaxon_pjrt.so")
            if hook is not None:
                mod.set_axon_ntff_profile_hook(hook)
        except Exception as e:  # pragma: no cover
            print(f"ntff hook setup failed: {e}")
    # keep artifacts local: no bucket upload in this environment
    bu.upload_artifacts = lambda tmpdir: tmpdir


if TRACE:
    _setup_trace_hooks()


def cdiv(a, b):
    return -(-a // b)


# ----------------------------------------------------------------------------
# host-side preprocessing
# ----------------------------------------------------------------------------

class Prep:
    pass


def preprocess(x, edge_index, batch_index, n_cores):
    pr = Prep()
    N = x.shape[0]
    G = int(batch_index.max()) + 1 if batch_index.shape[0] else 1
    # G must come from mol_features really; caller fixes pr.G
    row = np.asarray(edge_index[0], dtype=np.int64)
    col = np.asarray(edge_index[1], dtype=np.int64)
    bi = np.asarray(batch_index, dtype=np.int64)

    NT = cdiv(N, P)
    if NT % 2:
        NT += 1  # even tile count so the int16 split point is tile aligned
    Npad = NT * P
    assert Npad <= 65536, "int16 two-way split requires Npad/2 <= 32768"
    SPLIT = Npad // 2

    deg = np.bincount(col, minlength=N).astype(np.float64) + 1.0
    dinv = (1.0 / np.sqrt(deg)).astype(np.float32)

    pr.N, pr.NT, pr.Npad, pr.SPLIT = N, NT, Npad, SPLIT
    pr.dinv = dinv
    return pr, row, col, bi


def build_shards(pr, row, col, bi, G, n_cores):
    N = pr.N
    gpc = G // n_cores
    assert gpc * n_cores == G
    graph_start = np.searchsorted(bi, np.arange(G + 1))  # [G+1]; last == N
    shard_start = graph_start[np.arange(n_cores) * gpc]
    shard_end = graph_start[np.arange(1, n_cores + 1) * gpc]
    cw = (shard_end - shard_start).astype(np.int64)
    NW = max(1, int(cdiv(int(cw.max()), WIN)))
    NSH = NW * WIN
    pr.G, pr.gpc = G, gpc
    pr.graph_start = graph_start
    pr.shard_start, pr.shard_end, pr.cw = shard_start, shard_end, cw
    pr.NW, pr.NSH = NW, NSH

    # augmented edge list: original edges + self loops, S value = dinv[row]
    ar = np.concatenate([row, np.arange(N, dtype=np.int64)])
    ac = np.concatenate([col, np.arange(N, dtype=np.int64)])
    av = pr.dinv[ar]
    core_of = np.searchsorted(shard_start, ac, side="right") - 1

    per_core = []  # [core][stream] -> (r, d, v) sorted by d
    cnts = np.zeros((n_cores, 2, NSH), dtype=np.int64)
    for c in range(n_cores):
        m = core_of == c
        r_, d_, v_ = ar[m], ac[m] - shard_start[c], av[m]
        streams = []
        for s in range(2):
            sm = (r_ >= pr.SPLIT) if s else (r_ < pr.SPLIT)
            rs, dsv, vs = r_[sm], d_[sm], v_[sm]
            order = np.argsort(dsv, kind="stable")
            rs, dsv, vs = rs[order], dsv[order], vs[order]
            if s:
                rs = rs - pr.SPLIT
            cnts[c, s] = np.bincount(dsv, minlength=NSH)
            streams.append((rs.astype(np.int64), dsv, vs.astype(np.float32)))
        per_core.append(streams)
    pr.per_core_edges = per_core
    pr.cnts = cnts
    return pr


def build_schedule(pr, n_cores):
    """Shared (across cores) tile schedule per stream.

    Each tile: (window, col0, span, width, s_off) where width==WIN for the
    first tile of each window in stream 0 (full-width start=True matmul),
    else width==span. s_off is the running offset into the concatenated S
    array.
    """
    NW, NSH = pr.NW, pr.NSH
    schedules = []
    for s in range(2):
        cntmax = pr.cnts[:, s, :].max(axis=0)
        tiles = []
        s_off = 0
        for w in range(NW):
            d = w * WIN
            first = True
            while d < (w + 1) * WIN:
                c0 = d
                tot = 0
                while d < (w + 1) * WIN and tot + cntmax[d] <= P:
                    tot += cntmax[d]
                    d += 1
                assert d > c0, (
                    f"destination {d} has {cntmax[d]} edges > {P} on some core"
                )
                span = d - c0
                width = WIN if (first and s == 0) else span
                tiles.append(dict(w=w, col0=c0, span=span, width=width,
                                  s_off=s_off))
                s_off += width
                first = False
        schedules.append(dict(tiles=tiles, s_width=s_off,
                              n_tiles=len(tiles)))
    pr.schedules = schedules
    # per-window tile index ranges per stream
    for s in range(2):
        sch = schedules[s]
        win_ranges = []
        t0 = 0
        for w in range(pr.NW):
            t1 = t0
            while t1 < sch["n_tiles"] and sch["tiles"][t1]["w"] == w:
                t1 += 1
            win_ranges.append((t0, t1))
            t0 = t1
        sch["win_ranges"] = win_ranges
        sch["n_tiles_pad"] = cdiv(sch["n_tiles"], CHUNK_TILES) * CHUNK_TILES
    return pr


def wrap16(idx_flat):
    """Pack an int16 index stream into the [128, n/16] wrapped+replicated
    layout dma_gather expects (idx i lives at partition i%16, col i//16,
    replicated across the 8 16-partition groups)."""
    n = idx_flat.shape[0]
    assert n % 16 == 0
    a = idx_flat.reshape(n // 16, 16).T.astype(np.int16)  # [16, n/16]
    return np.tile(a, (8, 1))  # [128, n/16]


def build_core_data(pr, c):
    """Per-core gather index arrays and S matrices, following the shared
    schedule."""
    out = {}
    for s in range(2):
        sch = pr.schedules[s]
        rs, ds, vs = pr.per_core_edges[c][s]
        ntp = sch["n_tiles_pad"]
        idx = np.zeros(ntp * P, dtype=np.int16)
        S = np.zeros((P, sch["s_width"]), dtype=np.float32)
        # edges sorted by d; per tile take the contiguous run of its dsts
        starts = np.searchsorted(ds, np.arange(pr.NSH + 1))
        ht = pr.NT // 2  # tiles per half; g half layout row = (r%128)*ht + r//128
        for t, ti in enumerate(sch["tiles"]):
            lo = starts[ti["col0"]]
            hi = starts[ti["col0"] + ti["span"]]
            ne = hi - lo
            assert ne <= P
            if ne:
                r = rs[lo:hi]
                idx[t * P: t * P + ne] = (r % P) * ht + r // P
                S[np.arange(ne), ti["s_off"] + (ds[lo:hi] - ti["col0"])] = vs[lo:hi]
        out[f"idx{s}"] = wrap16(idx)
        out[f"S{s}"] = S.astype(BF16_NP)
    # dst-side norm, broadcast across partitions; zero in the padded tail
    dinvB = np.zeros((P, pr.NSH), dtype=np.float32)
    w = int(pr.cw[c])
    dinvB[:, :w] = pr.dinv[pr.shard_start[c]:pr.shard_end[c]][None, :]
    out["dinvB"] = dinvB
    return out


def build_pool_matrix(pr, c):
    """[P, NTS*gpc] one-hot node->graph indicator with 1/count folded in,
    laid out [p, t*gpc + j] for node-tile t, local graph j."""
    NTS = pr.NSH // P
    gpc = pr.gpc
    Sp = np.zeros((P, NTS * gpc), dtype=np.float32)
    g0 = c * gpc
    counts = np.maximum(
        (pr.graph_start[g0 + 1: g0 + gpc + 1] - pr.graph_start[g0: g0 + gpc]),
        1).astype(np.float64)
    # local graph id per local node
    w = int(pr.cw[c])
    bi_loc = np.searchsorted(pr.graph_start[g0: g0 + gpc + 1],
                             np.arange(pr.shard_start[c], pr.shard_end[c]),
                             side="right") - 1
    for n in range(w):
        t, p = divmod(n, P)
        j = bi_loc[n]
        Sp[p, t * gpc + j] = 1.0 / counts[j]
    return Sp


# ----------------------------------------------------------------------------
# device programs
# ----------------------------------------------------------------------------

def _emit_layer(nc, tc, pr, aps, h3_tile=None):
    """Emit transform + gather + scatter + evict for one GCN layer.

    aps: dict of dram APs (h1T, Wt, bvec, dinvB, S0, S1, idx0, idx1, g,
    out_shard or None when h3_tile given).
    If h3_tile is not None the evicted windows are written there (SBUF)
    instead of to HBM.
    """
    NT, NW, NSH = pr.NT, pr.NW, pr.NSH
    f32 = F32

    from contextlib import ExitStack
    stack = ExitStack()
    cpool = stack.enter_context(tc.tile_pool(name="Lconst", bufs=1))
    hpool = stack.enter_context(tc.tile_pool(name="Lh1", bufs=2))
    ptpool = stack.enter_context(tc.tile_pool(name="LpsumT", bufs=3,
                                              space="PSUM"))
    gspool = stack.enter_context(tc.tile_pool(name="Lgstage", bufs=3))
    ipool = stack.enter_context(tc.tile_pool(name="Lidx", bufs=1))
    gapool = stack.enter_context(tc.tile_pool(name="Lgath", bufs=3))
    spool = stack.enter_context(tc.tile_pool(name="Lswin", bufs=2))
    pwpool = stack.enter_context(tc.tile_pool(name="LpsumW", bufs=3,
                                              space="PSUM"))
    epool = stack.enter_context(tc.tile_pool(name="Levict", bufs=3))

    Wt_t = cpool.tile([P, P], BF16, tag="wt")
    nc.sync.dma_start(Wt_t[:], aps["Wt"])
    bvec_t = cpool.tile([P, 1], f32, tag="bv")
    nc.sync.dma_start(bvec_t[:], aps["bvec"])
    dinvB_t = cpool.tile([P, NSH], f32, tag="dinvB")
    nc.sync.dma_start(dinvB_t[:], aps["dinvB"])

    idx_t = []
    for s in range(2):
        ntp = pr.schedules[s]["n_tiles_pad"]
        t = ipool.tile([P, ntp * 8], I16, tag=f"idx{s}")
        nc.sync.dma_start(t[:], aps[f"idx{s}"])
        idx_t.append(t)

    # ---- transform: h = h1 @ W.T, node-major out -> g halves in HBM ------
    # g half layout: [SPLIT rows, 128] where flat row = (node%128)*ht + node//128
    # (ht tiles per half) -> per-partition contiguous writes.
    ht = NT // 2
    g_views = [aps["g0"].rearrange("(p t) f -> p t f", p=P),
               aps["g1"].rearrange("(p t) f -> p t f", p=P)]
    TCH = 28  # node tiles per h1 chunk (multiple of 4, divides ht=196)
    assert ht % 4 == 0
    for ch in range(cdiv(NT, TCH)):
        t0 = ch * TCH
        ntile = min(TCH, NT - t0)
        hc = hpool.tile([P, TCH * P], BF16, tag="h1c")
        nc.sync.dma_start(hc[:, : ntile * P],
                          aps["h1T"][:, t0 * P: (t0 + ntile) * P])
        for q in range(0, ntile, 4):
            ngrp = min(4, ntile - q)
            pt = ptpool.tile([P, 4 * P], f32, tag="ptrans")
            for j in range(ngrp):
                nc.tensor.matmul(
                    out=pt[:, j * P: (j + 1) * P],
                    lhsT=hc[:, (q + j) * P: (q + j + 1) * P],
                    rhs=Wt_t[:],
                    start=True, stop=True)
            st = gspool.tile([P, 4 * P], BF16, tag="gst")
            nc.vector.tensor_copy(out=st[:, : ngrp * P], in_=pt[:, : ngrp * P])
            # SBUF [p, (j f)] -> g_half[p, tloc+j, f]
            tg = t0 + q
            half = 0 if tg < ht else 1
            tloc = tg - half * ht
            assert tg + ngrp <= ht or tg >= ht, (tg, ngrp, ht)
            nc.sync.dma_start(
                g_views[half][:, tloc: tloc + ngrp, :],
                st[:, : ngrp * P].rearrange("p (j f) -> p j f", f=P))

    # ---- gather + matmul scatter + evict ---------------------------------
    halves = [aps["g0"], aps["g1"]]
    gath_tiles = [{}, {}]

    qctr = [0]

    def chunk_tile(s, cidx):
        if cidx not in gath_tiles[s]:
            t = gapool.tile([P, CHUNK_TILES, P], BF16, tag="gchunk")
            if os.environ.get("GCN_NO_GATHER"):
                nc.vector.memset(t[:], 0.0)
            else:
                nc.gpsimd.dma_gather(
                    t[:],
                    halves[s],
                    idx_t[s][:, cidx * (GATHER_CHUNK // 16):
                             (cidx + 1) * (GATHER_CHUNK // 16)],
                    GATHER_CHUNK, GATHER_CHUNK, P,
                    single_packet=(GATHER_CHUNK <= 512),
                    queue_num=qctr[0] % N_QUEUES)
                qctr[0] += 1
            gath_tiles[s][cidx] = t
        return gath_tiles[s][cidx]

    for w in range(NW):
        pw = pwpool.tile([P, WIN], f32, tag="pwin")
        s_sb = []
        for s in range(2):
            sch = pr.schedules[s]
            tr = sch["win_ranges"][w]
            w_lo = sch["tiles"][tr[0]]["s_off"]
            w_hi = (sch["tiles"][tr[1] - 1]["s_off"] +
                    sch["tiles"][tr[1] - 1]["width"]) if tr[1] > tr[0] else w_lo
            wwidth = w_hi - w_lo
            st = spool.tile([P, pr.max_swin[s]], BF16, tag=f"swin{s}")
            if wwidth:
                nc.sync.dma_start(st[:, :wwidth],
                                  aps[f"S{s}"][:, w_lo:w_hi])
            s_sb.append((st, w_lo))

        first = True
        n_mm = sum(pr.schedules[s]["win_ranges"][w][1] -
                   pr.schedules[s]["win_ranges"][w][0] for s in range(2))
        mm_i = 0
        for s in range(2):
            sch = pr.schedules[s]
            st, w_lo = s_sb[s]
            for t in range(*sch["win_ranges"][w]):
                ti = sch["tiles"][t]
                gt = chunk_tile(s, t // CHUNK_TILES)
                lhsT = gt[:, t % CHUNK_TILES, :]
                rhs = st[:, ti["s_off"] - w_lo: ti["s_off"] - w_lo + ti["width"]]
                out = pw[:, ti["col0"] - w * WIN:
                         ti["col0"] - w * WIN + ti["width"]]
                mm_i += 1
                nc.tensor.matmul(out=out, lhsT=lhsT, rhs=rhs,
                                 start=first, stop=(mm_i == n_mm),
                                 skip_group_check=True)
                first = False

        ev = epool.tile([P, WIN], f32, tag="ev")
        nc.vector.tensor_mul(out=ev[:], in0=pw[:],
                             in1=dinvB_t[:, w * WIN: (w + 1) * WIN])
        if h3_tile is not None:
            nc.scalar.activation(out=h3_tile[:, w * WIN: (w + 1) * WIN],
                                 in_=ev[:],
                                 func=mybir.ActivationFunctionType.Relu,
                                 bias=bvec_t[:, 0:1])
        else:
            ev2 = epool.tile([P, WIN], BF16, tag="ev2")
            nc.scalar.activation(out=ev2[:], in_=ev[:],
                                 func=mybir.ActivationFunctionType.Relu,
                                 bias=bvec_t[:, 0:1])
            nc.sync.dma_start(aps["out_shard"][:, w * WIN: (w + 1) * WIN],
                              ev2[:])

    stack.close()


def build_layer_program(pr, n_cores):
    nc = bacc.Bacc("TRN2", target_bir_lowering=False, debug=False,
                   num_devices=n_cores, num_swdge_queues=N_QUEUES)
    f32 = F32
    aps = {}
    aps["h1T"] = nc.dram_tensor("h1T", [P, pr.Npad], BF16,
                                kind="ExternalInput").ap()
    aps["Wt"] = nc.dram_tensor("Wt", [P, P], BF16, kind="ExternalInput").ap()
    aps["bvec"] = nc.dram_tensor("bvec", [P, 1], f32,
                                 kind="ExternalInput").ap()
    aps["dinvB"] = nc.dram_tensor("dinvB", [P, pr.NSH], f32,
                                  kind="ExternalInput").ap()
    for s in range(2):
        sch = pr.schedules[s]
        aps[f"S{s}"] = nc.dram_tensor(f"S{s}", [P, sch["s_width"]], BF16,
                                      kind="ExternalInput").ap()
        aps[f"idx{s}"] = nc.dram_tensor(f"idx{s}", [P, sch["n_tiles_pad"] * 8],
                                        I16, kind="ExternalInput").ap()
    aps["g0"] = nc.dram_tensor("g0", [pr.SPLIT, P], BF16,
                               kind="Internal").ap()
    aps["g1"] = nc.dram_tensor("g1", [pr.SPLIT, P], BF16,
                               kind="Internal").ap()
    aps["out_shard"] = nc.dram_tensor("out_shard", [P, pr.NSH], BF16,
                                      kind="ExternalOutput").ap()

    with tile.TileContext(nc) as tc:
        nc.gpsimd.load_library(library_config.mlp)
        _emit_layer(nc, tc, pr, aps)
    nc.compile()
    return nc  # noqa


def build_final_program(pr, n_cores):
    """Layer 3 + gcn_out linear + mean pool + MLP + predictor head."""
    nc = bacc.Bacc("TRN2", target_bir_lowering=False, debug=False,
                   num_devices=n_cores, num_swdge_queues=N_QUEUES)
    f32 = F32
    gpc = pr.gpc
    NTS = pr.NSH // P
    aps = {}
    aps["h1T"] = nc.dram_tensor("h1T", [P, pr.Npad], BF16,
                                kind="ExternalInput").ap()
    aps["Wt"] = nc.dram_tensor("Wt", [P, P], BF16, kind="ExternalInput").ap()
    aps["bvec"] = nc.dram_tensor("bvec", [P, 1], f32,
                                 kind="ExternalInput").ap()
    aps["dinvB"] = nc.dram_tensor("dinvB", [P, pr.NSH], f32,
                                  kind="ExternalInput").ap()
    for s in range(2):
        sch = pr.schedules[s]
        aps[f"S{s}"] = nc.dram_tensor(f"S{s}", [P, sch["s_width"]], BF16,
                                      kind="ExternalInput").ap()
        aps[f"idx{s}"] = nc.dram_tensor(f"idx{s}", [P, sch["n_tiles_pad"] * 8],
                                        I16, kind="ExternalInput").ap()
    aps["g0"] = nc.dram_tensor("g0", [pr.SPLIT, P], BF16,
                               kind="Internal").ap()
    aps["g1"] = nc.dram_tensor("g1", [pr.SPLIT, P], BF16,
                               kind="Internal").ap()

    # head weights (all pre-transposed host side to [fan_in, fan_out])
    aps["WoT"] = nc.dram_tensor("WoT", [P, P], BF16,
                                kind="ExternalInput").ap()
    aps["boB"] = nc.dram_tensor("boB", [P, P], f32, kind="ExternalInput").ap()
    aps["Spool"] = nc.dram_tensor("Spool", [P, NTS * gpc], f32,
                                  kind="ExternalInput").ap()
    aps["molT"] = nc.dram_tensor("molT", [256, gpc], f32,
                                 kind="ExternalInput").ap()
    aps["W1T"] = nc.dram_tensor("W1T", [256, 256], f32,
                                kind="ExternalInput").ap()
    aps["b1"] = nc.dram_tensor("b1", [P, 2], f32, kind="ExternalInput").ap()
    aps["W2T"] = nc.dram_tensor("W2T", [256, 256], f32,
                                kind="ExternalInput").ap()
    aps["b2"] = nc.dram_tensor("b2", [P, 2], f32, kind="ExternalInput").ap()
    aps["WmoT"] = nc.dram_tensor("WmoT", [256, 64], f32,
                                 kind="ExternalInput").ap()
    aps["bmo"] = nc.dram_tensor("bmo", [64, 1], f32,
                                kind="ExternalInput").ap()
    aps["P1T"] = nc.dram_tensor("P1T", [192, 256], f32,
                                kind="ExternalInput").ap()
    aps["pb1"] = nc.dram_tensor("pb1", [P, 2], f32, kind="ExternalInput").ap()
    aps["P2T"] = nc.dram_tensor("P2T", [256, 256], f32,
                                kind="ExternalInput").ap()
    aps["pb2"] = nc.dram_tensor("pb2", [P, 2], f32, kind="ExternalInput").ap()
    aps["oT"] = nc.dram_tensor("oT", [256, 1], f32,
                               kind="ExternalInput").ap()
    aps["ob"] = nc.dram_tensor("ob", [1, 1], f32, kind="ExternalInput").ap()
    aps["pred"] = nc.dram_tensor("pred", [1, gpc], f32,
                                 kind="ExternalOutput").ap()

    Relu = mybir.ActivationFunctionType.Relu
    Copy = mybir.ActivationFunctionType.Copy

    from contextlib import ExitStack
    with tile.TileContext(nc) as tc, ExitStack() as hstack:
        nc.gpsimd.load_library(library_config.mlp)
        perpool = hstack.enter_context(tc.tile_pool(name="Fpersist", bufs=1))
        h3 = perpool.tile([P, pr.NSH], BF16, tag="h3")
        _emit_layer(nc, tc, pr, aps, h3_tile=h3)

        hcpool = hstack.enter_context(tc.tile_pool(name="Fhconst", bufs=1))
        hppool = hstack.enter_context(tc.tile_pool(name="Fhpsum", bufs=2,
                                                   space="PSUM"))
        hspool = hstack.enter_context(tc.tile_pool(name="Fhsbuf", bufs=3))

        WoT_t = hcpool.tile([P, P], BF16, tag="WoT")
        nc.sync.dma_start(WoT_t[:], aps["WoT"])
        boB_t = hcpool.tile([P, P], f32, tag="boB")
        nc.sync.dma_start(boB_t[:], aps["boB"])
        Spool_t = hcpool.tile([P, NTS * gpc], f32, tag="Spool")
        nc.sync.dma_start(Spool_t[:], aps["Spool"])

        # gcn_out transform (node-major out) + mean pool matmul
        pool_ps = hppool.tile([P, gpc], f32, tag="poolps")
        for t in range(NTS):
            hp = hppool.tile([P, P], f32, tag="houtps")
            nc.tensor.matmul(out=hp[:], lhsT=h3[:, t * P: (t + 1) * P],
                             rhs=WoT_t[:], start=True, stop=True)
            hs = hspool.tile([P, P], f32, tag="houts")
            nc.vector.tensor_add(out=hs[:], in0=hp[:], in1=boB_t[:])
            nc.tensor.matmul(out=pool_ps[:], lhsT=hs[:],
                             rhs=Spool_t[:, t * gpc: (t + 1) * gpc],
                             start=(t == 0), stop=(t == NTS - 1))
        poolT = hcpool.tile([P, gpc], f32, tag="poolT")
        nc.scalar.activation(out=poolT[:], in_=pool_ps[:], func=Copy)

        # --- feature-major head, [*, gpc] ---------------------------------
        def load_w(name, rows, cols):
            tls = []
            for h in range(cdiv(rows, P)):
                r = min(P, rows - h * P)
                t = hcpool.tile([P, cols], f32, tag=f"{name}_{h}")
                nc.sync.dma_start(t[:r, :], aps[name][h * P: h * P + r, :])
                tls.append((t, r))
            return tls

        def load_b(name, rows):
            nh = cdiv(rows, P)
            t = hcpool.tile([P, nh], f32, tag=f"{name}_b")
            nc.sync.dma_start(t[:min(P, rows), :nh], aps[name])
            return t

        molT = load_w("molT", 256, gpc)
        W1T = load_w("W1T", 256, 256)
        b1 = load_b("b1", 256)
        W2T = load_w("W2T", 256, 256)
        b2 = load_b("b2", 256)
        WmoT = load_w("WmoT", 256, 64)
        bmo = load_b("bmo", 64)
        P1T = load_w("P1T", 192, 256)
        pb1 = load_b("pb1", 256)
        P2T = load_w("P2T", 256, 256)
        pb2 = load_b("pb2", 256)
        oT = load_w("oT", 256, 1)
        ob = load_b("ob", 1)

        def dense(x_parts, wts, bias_t, fan_out, act):
            """x_parts: list of (tile, rows) feature-major activations.
            wts: list of (tile, rows) chunks of [fan_in, fan_out] weights.
            Returns list of (tile, rows) output chunks."""
            outs = []
            for h in range(cdiv(fan_out, P)):
                fo = min(P, fan_out - h * P)
                ps = hppool.tile([P, gpc], f32, tag="headps")
                for k, (xt, xr) in enumerate(x_parts):
                    wt, wr = wts[k]
                    assert wr == xr
                    nc.tensor.matmul(
                        out=ps[:fo, :],
                        lhsT=wt[:xr, h * P: h * P + fo],
                        rhs=xt[:xr, :],
                        start=(k == 0), stop=(k == len(x_parts) - 1))
                ot = hspool.tile([P, gpc], f32, tag="heados")
                if act is None:  # bias only, no nonlinearity
                    nc.vector.tensor_scalar_add(out=ot[:fo, :],
                                                in0=ps[:fo, :],
                                                scalar1=bias_t[:fo, h: h + 1])
                else:
                    nc.scalar.activation(out=ot[:fo, :], in_=ps[:fo, :],
                                         func=act,
                                         bias=bias_t[:fo, h: h + 1])
                outs.append((ot, fo))
            return outs

        h2 = dense(molT, W1T, b1, 256, Relu)
        h2 = dense(h2, W2T, b2, 256, Relu)
        h2 = dense(h2, WmoT, bmo, 64, Relu)
        cat = [(poolT, P), h2[0]]
        p1 = dense(cat, P1T, pb1, 256, Relu)
        p2 = dense(p1, P2T, pb2, 256, Relu)
        o = dense(p2, oT, ob, 1, None)
        nc.sync.dma_start(aps["pred"], o[0][0][:1, :])
    nc.compile()
    return nc


# ----------------------------------------------------------------------------
# launch helpers
# ----------------------------------------------------------------------------

_RUNNERS = {}


def _build_runner(nc, n_cores):
    """AOT-compiled shard_map executable for `nc` (adapted from
    bass2jax.run_bass_via_pjrt, but compiled once and reusable)."""
    import jax
    from jax.experimental.shard_map import shard_map
    from jax.sharding import Mesh, PartitionSpec
    from concourse import bass2jax, mybir as mb

    bass2jax.install_neuronx_cc_hook()
    partition_name = (nc.partition_id_tensor.name
                      if nc.partition_id_tensor else None)
    in_names, out_names, out_avals, zero_shapes = [], [], [], []
    for alloc in nc.m.functions[0].allocations:
        if not isinstance(alloc, mb.MemoryLocationSet):
            continue
        name = alloc.memorylocations[0].name
        if alloc.kind == "ExternalInput":
            if name != partition_name:
                in_names.append(name)
        elif alloc.kind == "ExternalOutput":
            shape = tuple(alloc.tensor_shape)
            dtype = mb.dt.np(alloc.dtype)
            out_names.append(name)
            out_avals.append(jax.core.ShapedArray(shape, dtype))
            zero_shapes.append((shape, dtype))
    n_params = len(in_names)
    n_outs = len(out_avals)
    all_in_names = list(in_names) + list(out_names)
    if partition_name is not None:
        all_in_names.append(partition_name)
    donate = tuple(range(n_params, n_params + n_outs))

    def _body(*args):
        operands = list(args)
        if partition_name is not None:
            operands.append(bass2jax.partition_id_tensor())
        outs = bass2jax._bass_exec_p.bind(
            *operands,
            out_avals=tuple(out_avals),
            in_names=tuple(all_in_names),
            out_names=tuple(out_names),
            lowering_input_output_aliases=(),
            sim_require_finite=True,
            sim_require_nnan=True,
            nc=nc,
        )
        return tuple(outs)

    devices = jax.devices()[:n_cores]
    mesh = Mesh(np.asarray(devices), ("core",))
    in_specs = (PartitionSpec("core"),) * (n_params + n_outs)
    out_specs = (PartitionSpec("core"),) * n_outs
    jitted = jax.jit(
        shard_map(_body, mesh=mesh, in_specs=in_specs, out_specs=out_specs,
                  check_rep=False),
        donate_argnums=donate, keep_unused=True)

    # abstract args for AOT lowering (global = n_cores * per-core on axis 0)
    abstract = []
    shp = {}
    for alloc in nc.m.functions[0].allocations:
        if isinstance(alloc, mb.MemoryLocationSet) and alloc.kind in (
                "ExternalInput", "ExternalOutput"):
            shp[alloc.memorylocations[0].name] = (
                tuple(alloc.tensor_shape), mb.dt.np(alloc.dtype))
    for name in in_names:
        s, d = shp[name]
        abstract.append(jax.ShapeDtypeStruct((n_cores * s[0],) + s[1:], d))
    for s, d in zero_shapes:
        abstract.append(jax.ShapeDtypeStruct((n_cores * s[0],) + s[1:], d))
    compiled = jitted.lower(*abstract).compile()
    return dict(compiled=compiled, in_names=in_names, out_names=out_names,
                zero_shapes=zero_shapes, out_avals=out_avals)


def _run_launch(nc, in_maps, n_cores):
    if SIM_MODE:
        from concourse.bass_interp import MultiCoreSim
        sim = MultiCoreSim(nc, num_cores=n_cores)
        for cid, core in sim.cores.items():
            for k, v in in_maps[cid].items():
                core.tensor(k)[:] = v
        sim.simulate(check_with_hw=False)
        outs = []
        for cid in range(n_cores):
            core = sim.cores[cid]
            o = {}
            for alloc in nc.m.functions[0].allocations:
                if getattr(alloc, "kind", None) == "ExternalOutput":
                    name = alloc.memorylocations[0].name
                    o[name] = np.array(core.tensor(name))
            outs.append(o)
        return outs

    key = id(nc)
    if key not in _RUNNERS:
        _RUNNERS[key] = _build_runner(nc, n_cores)
    r = _RUNNERS[key]
    concat_in = [np.concatenate([np.asarray(in_maps[c][name])
                                 for c in range(n_cores)], axis=0)
                 for name in r["in_names"]]
    concat_zeros = [np.zeros((n_cores * s[0],) + s[1:], d)
                    for s, d in r["zero_shapes"]]

    def _exec():
        out = r["compiled"](*concat_in, *concat_zeros)
        import jax
        jax.block_until_ready(out)
        return out

    if TRACE:
        import tempfile
        from antenv.axon_hooks import get_axon_ntff_profile_hook
        hook = get_axon_ntff_profile_hook()
        tmpdir = tempfile.mkdtemp(prefix="gcn_ntff_")
        try:
            with hook(tmpdir, list(range(n_cores))):
                out_arrs = _exec()
            ns = _extract_exec_ns(nc, tmpdir, n_cores)
            if ns is not None:
                LAST_EXEC_NS.append(ns)
        except Exception as e:
            print(f"traced exec failed ({type(e).__name__}: {e}); "
                  "running untraced")
            out_arrs = _exec()
    else:
        out_arrs = _exec()

    outs = []
    for c in range(n_cores):
        o = {}
        for i, name in enumerate(r["out_names"]):
            a = np.asarray(out_arrs[i])
            per = a.reshape(n_cores, *r["out_avals"][i].shape)
            o[name] = per[c]
        outs.append(o)
    return outs


def _extract_exec_ns(nc, neff_dir, n_cores):
    """ntff -> perfetto -> max per-core exec time (same pipeline as
    bass_utils' axon trace path, kept local)."""
    try:
        import glob as _glob
        from concourse import bass_utils as bu
        import gauge.profiler
        from concourse._compat import FishPath
        ntffs = _glob.glob(os.path.join(neff_dir, "*_body*.ntff"))
        if not ntffs:
            print(f"no ntffs in {neff_dir}: {sorted(os.listdir(neff_dir))}")
            return None
        profile = gauge.profiler.Profile(
            profile_path=FishPath(neff_dir),
            kernel_dev_mode=True,
            profile_on_exit=False,
            bass_kernel=nc.m,
            offline_processing=True,
            fname="*_body*",
            metadata={"artifacts_path": neff_dir},
        )
        pres = bu._process_ntff_profile(
            profile, neff_dir, nc, list(range(n_cores)),
            list(range(n_cores)), False, {}, trace_events=False)
        print(f"  trace dir: {neff_dir}")
        return pres.exec_time_ns
    except Exception as e:
        import traceback
        traceback.print_exc()
        print(f"ntff processing failed: {e}")
        return None


def _layer_inmaps(pr, core_data, h1T, W, b, n_cores):
    Wt = np.ascontiguousarray(W.T).astype(BF16_NP)
    bvec = np.ascontiguousarray(b.reshape(P, 1)).astype(np.float32)
    maps = []
    for c in range(n_cores):
        cd = core_data[c]
        maps.append(dict(h1T=h1T, Wt=Wt, bvec=bvec, dinvB=cd["dinvB"],
                         S0=cd["S0"], S1=cd["S1"],
                         idx0=cd["idx0"], idx1=cd["idx1"]))
    return maps


def _assemble(pr, outs, n_cores):
    h1T = np.zeros((P, pr.Npad), dtype=BF16_NP)
    off = 0
    for c in range(n_cores):
        w = int(pr.cw[c])
        h1T[:, off:off + w] = outs[c]["out_shard"][:, :w]
        off += w
    return h1T


# program cache (avoid rebuilding for repeated kernel() calls)
_CACHE = {}


def gcn_forward(inputs, n_cores=8):
    if not SIM_MODE:
        _axon_reset()
    x = np.asarray(inputs["x"], dtype=np.float32)
    edge_index = np.asarray(inputs["edge_index"])
    batch_index = np.asarray(inputs["batch_index"])
    mol = np.asarray(inputs["mol_features"], dtype=np.float32)
    G = mol.shape[0]

    pr, row, col, bi = preprocess(x, edge_index, batch_index, n_cores)
    build_shards(pr, row, col, bi, G, n_cores)
    build_schedule(pr, n_cores)
    pr.max_swin = []
    for s in range(2):
        sch = pr.schedules[s]
        widths = []
        for w in range(pr.NW):
            lo, hi = sch["win_ranges"][w]
            widths.append(sum(t["width"] for t in sch["tiles"][lo:hi]))
        pr.max_swin.append(max(max(widths), 1))

    core_data = [build_core_data(pr, c) for c in range(n_cores)]

    key = (pr.Npad, pr.NSH, tuple(s["s_width"] for s in pr.schedules),
           tuple(s["n_tiles_pad"] for s in pr.schedules),
           tuple(tuple((t["col0"], t["width"], t["w"]) for t in s["tiles"])
                 for s in pr.schedules), G)
    if key not in _CACHE:
        _CACHE.clear()
        nc1 = build_layer_program(pr, n_cores)
        nc2 = build_final_program(pr, n_cores)
        _CACHE[key] = (nc1, nc2)
    nc1, nc2 = _CACHE[key]

    # layer 1+2 on program 1
    h1T = np.zeros((P, pr.Npad), dtype=BF16_NP)
    h1T[:, :pr.N] = np.ascontiguousarray(x.T).astype(BF16_NP)
    gcn_W = np.asarray(inputs["gcn_W"], dtype=np.float32)
    gcn_b = np.asarray(inputs["gcn_b"], dtype=np.float32)
    for L in range(2):
        maps = _layer_inmaps(pr, core_data, h1T, gcn_W[L], gcn_b[L], n_cores)
        outs = _run_launch(nc1, maps, n_cores)
        h1T = _assemble(pr, outs, n_cores)

    # layer 3 + head on program 2
    maps = _layer_inmaps(pr, core_data, h1T, gcn_W[2], gcn_b[2], n_cores)
    gpc = pr.gpc
    for c in range(n_cores):
        m = maps[c]
        m["WoT"] = np.ascontiguousarray(
            np.asarray(inputs["gcn_out_W"], dtype=np.float32).T).astype(
                BF16_NP)
        m["boB"] = np.tile(np.asarray(inputs["gcn_out_b"], dtype=np.float32),
                           (P, 1))
        m["Spool"] = build_pool_matrix(pr, c)
        m["molT"] = np.ascontiguousarray(
            mol[c * gpc:(c + 1) * gpc, :].T).astype(np.float32)
        mlp_W = np.asarray(inputs["mlp_W"], dtype=np.float32)
        mlp_b = np.asarray(inputs["mlp_b"], dtype=np.float32)
        m["W1T"] = np.ascontiguousarray(mlp_W[0].T)
        m["b1"] = np.ascontiguousarray(mlp_b[0].reshape(2, P).T)
        m["W2T"] = np.ascontiguousarray(mlp_W[1].T)
        m["b2"] = np.ascontiguousarray(mlp_b[1].reshape(2, P).T)
        m["WmoT"] = np.ascontiguousarray(
            np.asarray(inputs["mlp_out_W"], dtype=np.float32).T)
        m["bmo"] = np.asarray(inputs["mlp_out_b"],
                              dtype=np.float32).reshape(64, 1)
        m["P1T"] = np.ascontiguousarray(
            np.asarray(inputs["pred_W1"], dtype=np.float32).T)
        m["pb1"] = np.ascontiguousarray(
            np.asarray(inputs["pred_b1"], dtype=np.float32).reshape(2, P).T)
        m["P2T"] = np.ascontiguousarray(
            np.asarray(inputs["pred_W2"], dtype=np.float32).T)
        m["pb2"] = np.ascontiguousarray(
            np.asarray(inputs["pred_b2"], dtype=np.float32).reshape(2, P).T)
        m["oT"] = np.ascontiguousarray(
            np.asarray(inputs["out_W"], dtype=np.float32).T)
        m["ob"] = np.asarray(inputs["out_b"], dtype=np.float32).reshape(1, 1)
    outs = _run_launch(nc2, maps, n_cores)

    pred = np.concatenate([outs[c]["pred"][0] for c in range(n_cores)])
    return pred.reshape(G, 1).astype(np.float32)


def _axon_reset():
    try:
        import ctypes
        import jax
        jax.devices()  # client must be initialized for reset to reach it
        lib = ctypes.CDLL("/opt/axon/libaxon_pjrt.so")
        lib.axon_reset.restype = ctypes.c_int64
        return lib.axon_reset()
    except Exception:
        return None


def kernel(**inputs):
    LAST_EXEC_NS.clear()
    return gcn_forward(inputs, n_cores=8)
